# revision 18
# baseline (speedup 1.0000x reference)
"""
Single-head causal attention on 8 Trainium2 NeuronCores.

Problem: embeddings [8, 2048, 1024] fp32, Wq/Wk/Wv [1024, 128] fp32.
    q,k,v = x @ W{q,k,v};  wei = softmax(mask(q k^T * C^-0.5));  out = wei @ v
Sharding: pure data-parallel - one batch element per core, no collectives.

Host-side prep per core (numpy, layout/precision only): cast to fp16 and
build xw = [Wq|Wk|Wv | x^T] ([C, 384+T]).

Measured constraints this schedule is built around:
 - ~7us framework preamble before the main body can issue anything; the
   two HWDGE queues (sync, scalar) then stream ~150 GB/s each, so the
   4.75MB input is fully resident only ~16us later.  Input DMAs are
   2-csub 3D-AP transfers (565ns of engine time per trigger), split
   even/odd csub across the queues, in 4 column batches (W+ch0 | ch1 |
   ch2 | ch3) so data arrives in the order the chunk eras consume it.
 - HAM un-throttles only after ~3.4-6us of dense PE activity: one long
   accumulating warmup group runs while batch 1 lands.
 - The scalar engine's exp stream is (N+352)/1.2 ns per tile (~26us
   total); S tiles are emitted per-chunk right after that chunk's K/Q
   projections, with all other PE work (V proj, v_nat transposes, the
   previous chunk's PV burst) interleaved between S matmuls as filler.
 - A(ch) += P^T_j on DVE in fp16 (2x mode); A ships fp16, out^T ships
   fp16; host does the tiny denominator reduction and the divide.
"""

import numpy as np

B, T, C, H = 8, 2048, 1024, 128
N_CORES = 8
CHUNK = 512               # q-chunk width (one PSUM bank of fp32)
N_CHUNKS = T // CHUNK     # 4
N_CSUB = C // 128         # 8 contraction subtiles
KT_PER_CHUNK = CHUNK // 128
W_COLS = 3 * H            # 384: [Wq|Wk|Wv] prefix of each xw row
SCALE = float(C) ** -0.5  # 1/32, matches reference (embed-size scaling)
N_WARMUP = 10   # sized so the dense warmup ends ~when x-ch0 lands (~13us);
                # shorter leaves a PE idle gap that re-throttles HAM

_CACHE = {}


def _build_bass():
    import concourse.tile as tile
    from concourse import bacc, mybir
    from concourse.masks import make_identity

    fp16 = mybir.dt.float16
    fp32 = mybir.dt.float32
    Exp = mybir.ActivationFunctionType.Exp

    nc = bacc.Bacc("TRN2", target_bir_lowering=False, debug=False,
                   num_devices=N_CORES)

    # declared [csub, 128, cols] (same linear layout as [C, cols]) so the
    # paired-csub DMA source can be expressed as a 3D AP transpose
    xw_d = nc.dram_tensor("xw", [N_CSUB, 128, W_COLS + T], fp16,
                          kind="ExternalInput")
    # per-chunk combined output: [out^T | A] in one DMA
    oa_d = nc.dram_tensor("oa", [N_CHUNKS, 128, 2, CHUNK], fp16,
                          kind="ExternalOutput")

    with tile.TileContext(nc) as tc:
        with (
            tc.tile_pool(name="const", bufs=1) as constp,
            tc.tile_pool(name="work", bufs=8) as workp,
            tc.tile_pool(name="pt", bufs=32) as ptp,
        ):
            ident = constp.tile([128, 128], fp16, tag="ident")
            make_identity(nc, ident[:])
            scratch = constp.tile([128, CHUNK], fp16, tag="scratch")
            nc.gpsimd.memset(scratch[:], 0.0)

            # one SBUF home for the whole input: [128, csub, 384+2048]
            xw = constp.tile([128, N_CSUB, W_COLS + T], fp16, tag="xw")

            # input: multi-csub 3D-AP DMAs ([partition, csub, cols]); each
            # queue carries half the csubs.  Batch order = consumption
            # order: W first (gates every projection), then x chunk by
            # chunk; x-ch0 split in csub pairs so K0 can start accumulating
            # while the rest of batch 1 streams.
            col_batches = [(0, W_COLS, 4), (W_COLS, W_COLS + CHUNK, 2)] + [
                (W_COLS + ch * CHUNK, W_COLS + (ch + 1) * CHUNK, 4)
                for ch in range(1, N_CHUNKS)]
            for lo, hi, step in col_batches:
                for c0, eng in ((0, nc.sync), (4, nc.scalar)):
                    for c in range(c0, c0 + 4, step):
                        eng.dma_start(
                            out=xw[:, c:c + step, lo:hi],
                            in_=xw_d.ap()[c:c + step, :, lo:hi]
                                .transpose([1, 0, 2]))

            def w_sl(c, which):
                return xw[:, c, which * H:(which + 1) * H]

            def x_sl(c, ch):
                return xw[:, c, W_COLS + ch * CHUNK:W_COLS + (ch + 1) * CHUNK]

            qT = constp.tile([128, T], fp16, tag="qT")
            kT = constp.tile([128, T], fp16, tag="kT")
            vT = constp.tile([128, T], fp16, tag="vT")
            v_nat = constp.tile([128, T], fp16, tag="v_nat")

            with (
                tc.tile_pool(name="pproj", bufs=2, space="PSUM") as psproj,
                tc.tile_pool(name="pvt", bufs=1, space="PSUM") as psvt,
                tc.tile_pool(name="ps_s", bufs=3, space="PSUM") as pss,
                tc.tile_pool(name="ps_o", bufs=2, space="PSUM") as pso,
            ):
                # dense HAM warm-up (one accumulation group: back-to-back
                # columns, no per-MM drain gaps) while batch 1 lands
                warm_ps = pso.tile([128, CHUNK], fp32, tag="o")
                for i in range(N_WARMUP):
                    nc.tensor.matmul(warm_ps[:], ident[:], scratch[:],
                                     start=(i == 0), stop=(i == N_WARMUP - 1))

                def proj_mms(which, ch):
                    # lazy PSUM alloc: pool ring order must match engine
                    # usage order
                    box = {}

                    def mm(c):
                        if c == 0:
                            box["ps"] = psproj.tile(
                                [128, CHUNK], fp32, tag="proj",
                                name=f"proj{which}_{ch}")
                        nc.tensor.matmul(box["ps"][:], w_sl(c, which),
                                         x_sl(c, ch),
                                         start=(c == 0),
                                         stop=(c == N_CSUB - 1))

                    def cast():
                        dstT = (qT, kT, vT)[which]
                        cs = slice(ch * CHUNK, (ch + 1) * CHUNK)
                        nc.vector.tensor_copy(dstT[:, cs], box["ps"][:])

                    for c in range(N_CSUB):
                        yield lambda c=c: mm(c)
                    yield cast

                def transp_mms(ch):
                    for j in range(ch * KT_PER_CHUNK,
                                   (ch + 1) * KT_PER_CHUNK):
                        def one(j=j):
                            js = slice(j * 128, (j + 1) * 128)
                            psv = psvt.tile([128, 128], fp16, tag="vt")
                            nc.tensor.transpose(psv[:], vT[:, js], ident[:])
                            nc.vector.tensor_copy(v_nat[:, js], psv[:])
                        yield one

                def tile_geom(ch, j):
                    d = j - ch * KT_PER_CHUNK
                    q0 = ch * CHUNK + (128 * d if d >= 0 else 0)
                    n = (ch + 1) * CHUNK - q0
                    return d, q0, n, q0 - ch * CHUNK

                oa_tiles = {}   # [128, 2, CHUNK]: [:,0,:]=out^T, [:,1,:]=A
                pts = {}

                def attention_s(ch, j):
                    d, q0, n, lo = tile_geom(ch, j)
                    if ch not in oa_tiles:
                        oa_tiles[ch] = workp.tile([128, 2, CHUNK], fp16,
                                                  tag="oa",
                                                  name=f"oa_sb{ch}")
                    s_ps = pss.tile([128, n], fp32, tag="s")
                    nc.tensor.matmul(s_ps[:], kT[:, j * 128:(j + 1) * 128],
                                     qT[:, q0:(ch + 1) * CHUNK],
                                     start=True, stop=True)
                    pt = ptp.tile([128, n], fp16, tag="pt")
                    nc.scalar.activation(pt[:], s_ps[:], Exp, scale=SCALE)
                    if d >= 0:
                        nc.gpsimd.affine_select(
                            out=pt[:, 0:128], in_=pt[:, 0:128],
                            compare_op=mybir.AluOpType.is_ge,
                            fill=0.0, base=0,
                            pattern=[[1, 128]], channel_multiplier=-1)
                    a_sb = oa_tiles[ch][:, 1, :]
                    if j == 0:
                        nc.vector.tensor_copy(a_sb, pt[:])
                    else:
                        nc.vector.tensor_add(a_sb[:, lo:], a_sb[:, lo:],
                                             pt[:])
                    pts[(ch, j)] = pt

                def pv_out_mms(ch):
                    n_j = (ch + 1) * KT_PER_CHUNK
                    box = {}

                    for j in range(n_j):
                        def one(j=j):
                            if j == 0:
                                box["o"] = pso.tile([128, CHUNK], fp32,
                                                    tag="o", name=f"o_ps{ch}")
                            _, _, _, lo = tile_geom(ch, j)
                            nc.tensor.matmul(
                                box["o"][:, lo:],
                                v_nat[:, j * 128:(j + 1) * 128],
                                pts.pop((ch, j))[:],
                                start=(j == 0), stop=(j == n_j - 1),
                                skip_group_check=True)
                        yield one

                    def out():
                        # fp16 out^T into the combined tile; one DMA ships
                        # [out^T | A]; host divides in fp32
                        nc.vector.tensor_copy(oa_tiles[ch][:, 0, :],
                                              box["o"][:])
                        nc.sync.dma_start(out=oa_d.ap()[ch],
                                          in_=oa_tiles[ch][:])
                    yield out

                def era(ch, filler):
                    """K,Q proj inline; S tiles with filler interleaved."""
                    for f in proj_mms(1, ch):   # K
                        f()
                    for f in proj_mms(0, ch):   # Q
                        f()
                    n_s = (ch + 1) * KT_PER_CHUNK
                    n_f = len(filler)
                    emitted = 0
                    for j in range(n_s):
                        attention_s(ch, j)
                        want = round(n_f * (j + 1) / n_s)
                        while emitted < want:
                            filler[emitted]()
                            emitted += 1

                # ---- schedule: chunk eras in DMA-arrival order ----
                era(0, list(proj_mms(2, 0)) + list(transp_mms(0)))
                era(1, list(proj_mms(2, 1)) + list(transp_mms(1))
                        + list(pv_out_mms(0)))
                era(2, list(proj_mms(2, 2)) + list(transp_mms(2))
                        + list(pv_out_mms(1)))
                era(3, list(proj_mms(2, 3)) + list(transp_mms(3))
                        + list(pv_out_mms(2)))
                for f in pv_out_mms(3):
                    f()

    nc.compile()
    return nc


def _get_nc():
    if "nc" not in _CACHE:
        _CACHE["nc"] = _build_bass()
    return _CACHE["nc"]


LAST_RESULTS = None


def kernel(embeddings: np.ndarray, Wq: np.ndarray, Wk: np.ndarray,
           Wv: np.ndarray) -> np.ndarray:
    from concourse.bass_utils import run_bass_kernel_spmd
    import os

    nc = _get_nc()
    x16 = np.asarray(embeddings, dtype=np.float32).astype(np.float16)
    w16 = np.concatenate(
        [np.asarray(w, dtype=np.float32).astype(np.float16)
         for w in (Wq, Wk, Wv)], axis=1)          # [C, 3H]
    in_maps = [{"xw": np.ascontiguousarray(
        np.concatenate([w16, x16[b].T], axis=1)).reshape(
            N_CSUB, 128, W_COLS + T)} for b in range(B)]

    trace = bool(int(os.environ.get("KERNEL_TRACE", "0")))
    res = run_bass_kernel_spmd(nc, in_maps, core_ids=list(range(N_CORES)),
                               trace=trace)
    global LAST_RESULTS
    LAST_RESULTS = res

    out = np.empty((B, T, H), dtype=np.float32)
    for b in range(B):
        oa = res.results[b]["oa"]  # [N_CHUNKS, 128, 2, CHUNK]
        oT = np.concatenate(
            [oa[ch][:, 0, :].astype(np.float32) for ch in range(N_CHUNKS)],
            axis=1)
        l = np.concatenate(
            [oa[ch][:, 1, :].astype(np.float32).sum(axis=0)
             for ch in range(N_CHUNKS)])
        out[b] = (oT / l[None, :]).T
    return out


# revision 19
# speedup vs baseline: 1.0402x; 1.0402x over previous
"""
Single-head causal attention on 8 Trainium2 NeuronCores.

Problem: embeddings [8, 2048, 1024] fp32, Wq/Wk/Wv [1024, 128] fp32.
    q,k,v = x @ W{q,k,v};  wei = softmax(mask(q k^T * C^-0.5));  out = wei @ v
Sharding: pure data-parallel - one batch element per core, no collectives.

Host-side prep per core (numpy, layout/precision only): cast to fp16 and
build xw = [Wq|Wk|Wv | x^T] ([C, 384+T]).

Measured constraints this schedule is built around:
 - ~7us framework preamble before the main body can issue anything; the
   two HWDGE queues (sync, scalar) then stream ~150 GB/s each, so the
   4.75MB input is fully resident only ~16us later.  Input DMAs are
   2-csub 3D-AP transfers (565ns of engine time per trigger), split
   even/odd csub across the queues, in 4 column batches (W+ch0 | ch1 |
   ch2 | ch3) so data arrives in the order the chunk eras consume it.
 - HAM un-throttles only after ~3.4-6us of dense PE activity: one long
   accumulating warmup group runs while batch 1 lands.
 - The scalar engine's exp stream is (N+352)/1.2 ns per tile (~26us
   total); S tiles are emitted per-chunk right after that chunk's K/Q
   projections, with all other PE work (V proj, v_nat transposes, the
   previous chunk's PV burst) interleaved between S matmuls as filler.
 - A(ch) += P^T_j on DVE in fp16 (2x mode); A ships fp16, out^T ships
   fp16; host does the tiny denominator reduction and the divide.
"""

import numpy as np

B, T, C, H = 8, 2048, 1024, 128
N_CORES = 8
CHUNK = 512               # q-chunk width (one PSUM bank of fp32)
N_CHUNKS = T // CHUNK     # 4
N_CSUB = C // 128         # 8 contraction subtiles
KT_PER_CHUNK = CHUNK // 128
W_COLS = 3 * H            # 384: [Wq|Wk|Wv] prefix of each xw row
SCALE = float(C) ** -0.5  # 1/32, matches reference (embed-size scaling)
N_WARMUP = 10   # sized so the dense warmup ends ~when x-ch0 lands (~13us);
                # shorter leaves a PE idle gap that re-throttles HAM

_CACHE = {}


def _build_bass():
    import concourse.tile as tile
    from concourse import bacc, mybir
    from concourse.masks import make_identity

    fp16 = mybir.dt.float16
    fp32 = mybir.dt.float32
    Exp = mybir.ActivationFunctionType.Exp

    nc = bacc.Bacc("TRN2", target_bir_lowering=False, debug=False,
                   num_devices=N_CORES)

    # declared [csub, 128, cols] (same linear layout as [C, cols]) so the
    # paired-csub DMA source can be expressed as a 3D AP transpose
    xw_d = nc.dram_tensor("xw", [N_CSUB, 128, W_COLS + T], fp16,
                          kind="ExternalInput")
    # per-chunk combined output: [out^T | A] in one DMA
    oa_d = nc.dram_tensor("oa", [N_CHUNKS, 128, 2, CHUNK], fp16,
                          kind="ExternalOutput")

    with tile.TileContext(nc) as tc:
        with (
            tc.tile_pool(name="const", bufs=1) as constp,
            tc.tile_pool(name="work", bufs=8) as workp,
            tc.tile_pool(name="pt", bufs=32) as ptp,
        ):
            ident = constp.tile([128, 128], fp16, tag="ident")
            make_identity(nc, ident[:])
            scratch = constp.tile([128, CHUNK], fp16, tag="scratch")
            nc.gpsimd.memset(scratch[:], 0.0)

            # one SBUF home for the whole input: [128, csub, 384+2048]
            xw = constp.tile([128, N_CSUB, W_COLS + T], fp16, tag="xw")

            # input: multi-csub 3D-AP DMAs ([partition, csub, cols]); each
            # queue carries half the csubs.  Batch order = consumption
            # order: W first (gates every projection), then x chunk by
            # chunk; x-ch0 split in csub pairs so K0 can start accumulating
            # while the rest of batch 1 streams.
            col_batches = [(0, W_COLS + CHUNK, 2)] + [
                (W_COLS + ch * CHUNK, W_COLS + (ch + 1) * CHUNK, 4)
                for ch in range(1, N_CHUNKS)]
            for lo, hi, step in col_batches:
                for c0, eng in ((0, nc.sync), (4, nc.scalar)):
                    for c in range(c0, c0 + 4, step):
                        eng.dma_start(
                            out=xw[:, c:c + step, lo:hi],
                            in_=xw_d.ap()[c:c + step, :, lo:hi]
                                .transpose([1, 0, 2]))

            def w_sl(c, which):
                return xw[:, c, which * H:(which + 1) * H]

            def x_sl(c, ch):
                return xw[:, c, W_COLS + ch * CHUNK:W_COLS + (ch + 1) * CHUNK]

            qT = constp.tile([128, T], fp16, tag="qT")
            kT = constp.tile([128, T], fp16, tag="kT")
            vT = constp.tile([128, T], fp16, tag="vT")
            v_nat = constp.tile([128, T], fp16, tag="v_nat")

            with (
                tc.tile_pool(name="pproj", bufs=2, space="PSUM") as psproj,
                tc.tile_pool(name="pvt", bufs=1, space="PSUM") as psvt,
                tc.tile_pool(name="ps_s", bufs=3, space="PSUM") as pss,
                tc.tile_pool(name="ps_o", bufs=2, space="PSUM") as pso,
            ):
                # dense HAM warm-up (one accumulation group: back-to-back
                # columns, no per-MM drain gaps) while batch 1 lands
                warm_ps = pso.tile([128, CHUNK], fp32, tag="o")
                for i in range(N_WARMUP):
                    nc.tensor.matmul(warm_ps[:], ident[:], scratch[:],
                                     start=(i == 0), stop=(i == N_WARMUP - 1))

                def proj_mms(which, ch):
                    # lazy PSUM alloc: pool ring order must match engine
                    # usage order
                    box = {}

                    def mm(c):
                        if c == 0:
                            box["ps"] = psproj.tile(
                                [128, CHUNK], fp32, tag="proj",
                                name=f"proj{which}_{ch}")
                        nc.tensor.matmul(box["ps"][:], w_sl(c, which),
                                         x_sl(c, ch),
                                         start=(c == 0),
                                         stop=(c == N_CSUB - 1))

                    def cast():
                        dstT = (qT, kT, vT)[which]
                        cs = slice(ch * CHUNK, (ch + 1) * CHUNK)
                        nc.vector.tensor_copy(dstT[:, cs], box["ps"][:])

                    for c in range(N_CSUB):
                        yield lambda c=c: mm(c)
                    yield cast

                def transp_mms(ch):
                    for j in range(ch * KT_PER_CHUNK,
                                   (ch + 1) * KT_PER_CHUNK):
                        def one(j=j):
                            js = slice(j * 128, (j + 1) * 128)
                            psv = psvt.tile([128, 128], fp16, tag="vt")
                            nc.tensor.transpose(psv[:], vT[:, js], ident[:])
                            nc.vector.tensor_copy(v_nat[:, js], psv[:])
                        yield one

                def tile_geom(ch, j):
                    d = j - ch * KT_PER_CHUNK
                    q0 = ch * CHUNK + (128 * d if d >= 0 else 0)
                    n = (ch + 1) * CHUNK - q0
                    return d, q0, n, q0 - ch * CHUNK

                oa_tiles = {}   # [128, 2, CHUNK]: [:,0,:]=out^T, [:,1,:]=A
                pts = {}

                def attention_s(ch, j):
                    d, q0, n, lo = tile_geom(ch, j)
                    if ch not in oa_tiles:
                        oa_tiles[ch] = workp.tile([128, 2, CHUNK], fp16,
                                                  tag="oa",
                                                  name=f"oa_sb{ch}")
                    s_ps = pss.tile([128, n], fp32, tag="s")
                    nc.tensor.matmul(s_ps[:], kT[:, j * 128:(j + 1) * 128],
                                     qT[:, q0:(ch + 1) * CHUNK],
                                     start=True, stop=True)
                    pt = ptp.tile([128, n], fp16, tag="pt")
                    nc.scalar.activation(pt[:], s_ps[:], Exp, scale=SCALE)
                    if d >= 0:
                        nc.gpsimd.affine_select(
                            out=pt[:, 0:128], in_=pt[:, 0:128],
                            compare_op=mybir.AluOpType.is_ge,
                            fill=0.0, base=0,
                            pattern=[[1, 128]], channel_multiplier=-1)
                    a_sb = oa_tiles[ch][:, 1, :]
                    if j == 0:
                        nc.vector.tensor_copy(a_sb, pt[:])
                    else:
                        nc.vector.tensor_add(a_sb[:, lo:], a_sb[:, lo:],
                                             pt[:])
                    pts[(ch, j)] = pt

                def pv_out_mms(ch):
                    n_j = (ch + 1) * KT_PER_CHUNK
                    box = {}

                    for j in range(n_j):
                        def one(j=j):
                            if j == 0:
                                box["o"] = pso.tile([128, CHUNK], fp32,
                                                    tag="o", name=f"o_ps{ch}")
                            _, _, _, lo = tile_geom(ch, j)
                            nc.tensor.matmul(
                                box["o"][:, lo:],
                                v_nat[:, j * 128:(j + 1) * 128],
                                pts.pop((ch, j))[:],
                                start=(j == 0), stop=(j == n_j - 1),
                                skip_group_check=True)
                        yield one

                    def out():
                        # fp16 out^T into the combined tile; one DMA ships
                        # [out^T | A]; host divides in fp32
                        nc.vector.tensor_copy(oa_tiles[ch][:, 0, :],
                                              box["o"][:])
                        nc.sync.dma_start(out=oa_d.ap()[ch],
                                          in_=oa_tiles[ch][:])
                    yield out

                def era(ch, filler):
                    """K,Q proj inline; S tiles with filler interleaved."""
                    for f in proj_mms(1, ch):   # K
                        f()
                    for f in proj_mms(0, ch):   # Q
                        f()
                    n_s = (ch + 1) * KT_PER_CHUNK
                    n_f = len(filler)
                    emitted = 0
                    for j in range(n_s):
                        attention_s(ch, j)
                        want = round(n_f * (j + 1) / n_s)
                        while emitted < want:
                            filler[emitted]()
                            emitted += 1

                # ---- schedule: chunk eras in DMA-arrival order ----
                era(0, list(proj_mms(2, 0)) + list(transp_mms(0)))
                era(1, list(proj_mms(2, 1)) + list(transp_mms(1))
                        + list(pv_out_mms(0)))
                era(2, list(proj_mms(2, 2)) + list(transp_mms(2))
                        + list(pv_out_mms(1)))
                era(3, list(proj_mms(2, 3)) + list(transp_mms(3))
                        + list(pv_out_mms(2)))
                for f in pv_out_mms(3):
                    f()

    nc.compile()
    return nc


def _get_nc():
    if "nc" not in _CACHE:
        _CACHE["nc"] = _build_bass()
    return _CACHE["nc"]


LAST_RESULTS = None


def kernel(embeddings: np.ndarray, Wq: np.ndarray, Wk: np.ndarray,
           Wv: np.ndarray) -> np.ndarray:
    from concourse.bass_utils import run_bass_kernel_spmd
    import os

    nc = _get_nc()
    x16 = np.asarray(embeddings, dtype=np.float32).astype(np.float16)
    w16 = np.concatenate(
        [np.asarray(w, dtype=np.float32).astype(np.float16)
         for w in (Wq, Wk, Wv)], axis=1)          # [C, 3H]
    in_maps = [{"xw": np.ascontiguousarray(
        np.concatenate([w16, x16[b].T], axis=1)).reshape(
            N_CSUB, 128, W_COLS + T)} for b in range(B)]

    trace = bool(int(os.environ.get("KERNEL_TRACE", "0")))
    res = run_bass_kernel_spmd(nc, in_maps, core_ids=list(range(N_CORES)),
                               trace=trace)
    global LAST_RESULTS
    LAST_RESULTS = res

    out = np.empty((B, T, H), dtype=np.float32)
    for b in range(B):
        oa = res.results[b]["oa"]  # [N_CHUNKS, 128, 2, CHUNK]
        oT = np.concatenate(
            [oa[ch][:, 0, :].astype(np.float32) for ch in range(N_CHUNKS)],
            axis=1)
        l = np.concatenate(
            [oa[ch][:, 1, :].astype(np.float32).sum(axis=0)
             for ch in range(N_CHUNKS)])
        out[b] = (oT / l[None, :]).T
    return out
